# revision 22
# baseline (speedup 1.0000x reference)
"""BoundaryFluxAttention TRN2 kernel (v2 — lag-D dense-PE schedule).

Distribution (8 cores): data-parallel over batch (B=2) x tensor-parallel over
heads (16 heads -> 4 groups of 4). Core c handles batch c//4, head group c%4.
Each core computes a partial y_c = softmax-attention(its 4 heads) @ W_out rows
for those heads; the host sums the 4 partials per batch and adds b_out.

Design notes (from trace analysis of v1):
- PE is the critical path (~200us of matmul work/core vs ~171us of exp work on
  ACT). Every PE instruction must find its deps satisfied when the in-order
  queue reaches it, or the engine idles.
- The AV-accumulate (stage D) therefore runs ONE GROUP BEHIND the S/exp
  stream: exp outputs (P tiles) buffer in a 20-deep SBUF ring, so D's sem
  waits are satisfied ~16 slots in advance.
- Projections (A: Q/K, B: V) and out-proj (E) are emitted as deadline-ordered
  filler between the S/D pairs of each slot.
- All matmul operands bf16 (halves weight-load time + DMA). hd^-0.5 is folded
  into the exp's scale operand; boundary bias is the per-partition bias AP.
- Softmax denominators: ones column in vsb accumulates in the same matmul as
  O'^T (row 64). The denom row shifts to partition 0 via a tiny SBUF->SBUF
  DMA (DVE lane ops cannot cross partitions on HW), then
  reciprocal_approx_fast on DVE and partition_broadcast on the idle GPSIMD
  engine; ACT does only exp: one table-set, zero table reloads.
- PSUM: psS 2x[128,1024] (4 banks) + psO {osA,osB} (2) + psG 2x[128,512] (2).

Measured on HW (trace mode): 340us (v1 baseline) -> 212us; exp stream runs at
its ~1.1us/tile ACT floor for windows 2-7; remaining cost is the phase-1
projection bulge (~25us), startup (~12us) and the epilogue tail (~11us).
"""

import numpy as np

import concourse.bass as bass  # noqa: F401
import concourse.mybir as mybir
import concourse.tile as tile
from concourse import bacc

F32 = mybir.dt.float32
F32R = mybir.dt.float32r
BF16 = mybir.dt.bfloat16
EXP = mybir.ActivationFunctionType.Exp

T = 2048
D = 1024
HPC = 4          # heads per core
HD = 64
NKB = T // 128   # 16 k blocks of 128
NQB = T // 512   # 4 q blocks of 512
NCH = D // 128   # 8 contraction chunks
NGR = 8          # (qb, pi) groups
SCALE = HD ** -0.5
BIAS_COEF = 0.1

_NC_CACHE = {}


def _build_nc(with_qkv_bias=False):
    nc = bacc.Bacc("TRN2", target_bir_lowering=False)

    xt_d = nc.declare_dram_parameter("xt", [D, T], BF16, isOutput=False)
    wqk_d = nc.declare_dram_parameter("wqk", [D, 512], BF16, isOutput=False)
    bqk_d = nc.declare_dram_parameter("bqk", [1, 512], BF16, isOutput=False)
    wv_d = nc.declare_dram_parameter("wv", [D, 256], BF16, isOutput=False)
    bv_d = nc.declare_dram_parameter("bv", [1, 256], BF16, isOutput=False)
    wo_d = nc.declare_dram_parameter("wo", [256, D], BF16, isOutput=False)
    bs_d = nc.declare_dram_parameter("bs", [128, NKB], F32, isOutput=False)
    ones_d = nc.declare_dram_parameter("ones", [1, 512], BF16, isOutput=False)
    onesr_d = nc.declare_dram_parameter("onesr", [1, 64], F32R, isOutput=False)
    y_d = nc.declare_dram_parameter("y", [T, D], F32, isOutput=True)

    with tile.TileContext(nc) as tc:
        with (
            tc.tile_pool(name="const", bufs=1) as constp,
            tc.tile_pool(name="wts", bufs=1) as wts,
            tc.tile_pool(name="big", bufs=1) as bigp,
            tc.tile_pool(name="pt", bufs=20) as ptp,
            tc.tile_pool(name="stg", bufs=3) as stgp,
            tc.tile_pool(name="rc", bufs=2) as rcp,
            tc.tile_pool(name="rb", bufs=2) as rbp,
            tc.tile_pool(name="ysb", bufs=3) as ypool,
            tc.tile_pool(name="psS", bufs=2, space="PSUM") as psS,
            tc.tile_pool(name="psO", bufs=1, space="PSUM") as psO,
            tc.tile_pool(name="psG", bufs=2, space="PSUM") as psG,
        ):
            # ---- weights: per-chunk DMAs so A's first matmuls start early --
            wqk_chunks = wqk_d.rearrange("(c p) n -> c p n", p=128)
            wqk_sb = wts.tile([128, NCH, 512], BF16, tag="wqk")
            for c in range(NCH):
                nc.scalar.dma_start(wqk_sb[:, c, :], wqk_chunks[c])
            bs_sb = constp.tile([128, NKB], F32, tag="bs")
            nc.scalar.dma_start(bs_sb[:], bs_d[:])
            wv_sb = wts.tile([128, NCH, 256], BF16, tag="wv")
            nc.scalar.dma_start(wv_sb[:], wv_d.rearrange("(c p) n -> p c n", p=128))
            wo_sb = wts.tile([128, 2, D], BF16, tag="wo")
            nc.scalar.dma_start(wo_sb[:], wo_d.rearrange("(c p) n -> p c n", p=128))
            if with_qkv_bias:
                bqk_sb = wts.tile([1, 512], BF16, tag="bqk")
                nc.scalar.dma_start(bqk_sb[:], bqk_d[:])
                bv_sb = wts.tile([1, 256], BF16, tag="bv")
                nc.scalar.dma_start(bv_sb[:], bv_d[:])
                ones = constp.tile([1, 512], BF16, tag="ones")
                nc.scalar.dma_start(ones[:], ones_d[:])

            # x^T chunks, tb-major so tb0 lands first; split across the sync
            # and gpsimd DMA queues for parallel pull
            xT = bigp.tile([128, NCH, T], BF16, tag="xT")
            xt_chunks = xt_d.rearrange("(c p) t -> c p t", p=128)
            for tb in range(4):
                for c in range(NCH):
                    eng = nc.sync if c % 2 == 0 else nc.gpsimd
                    eng.dma_start(
                        xT[:, c, tb * 512:(tb + 1) * 512],
                        xt_chunks[c][:, tb * 512:(tb + 1) * 512],
                    )

            qkt = [
                bigp.tile([128, T], BF16, tag=f"qkt{db}", name=f"qkt{db}")
                for db in range(4)
            ]
            vsb = bigp.tile([128, NKB, HPC, 65], BF16, tag="vsb", name="vsb_v2")
            nc.vector.memset(vsb[:], 1.0)
            ot = [
                bigp.tile([128, T], BF16, tag=f"ot{pi}", name=f"ot{pi}")
                for pi in range(2)
            ]
            y_rows = y_d.rearrange("(n p) d -> n p d", p=128)

            # ---------------- emission units ----------------
            def a_unit(tb, db):
                ps = psG.tile([128, 512], F32, tag="gp", name=f"qk{tb}_{db}")
                for c in range(NCH):
                    nc.tensor.matmul(
                        ps[:],
                        wqk_sb[:, c, db * 128:(db + 1) * 128],
                        xT[:, c, tb * 512:(tb + 1) * 512],
                        start=(c == 0),
                        stop=(not with_qkv_bias and c == NCH - 1),
                    )
                if with_qkv_bias:
                    nc.tensor.matmul(
                        ps[:],
                        bqk_sb[0:1, db * 128:(db + 1) * 128],
                        ones[0:1, :],
                        start=False,
                        stop=True,
                    )
                nc.vector.tensor_copy(qkt[db][:, tb * 512:(tb + 1) * 512], ps[:])

            def b_unit(kb):
                ps = psG.tile([128, 512], F32, tag="gp", name=f"v{kb}")
                for c in range(NCH):
                    nc.tensor.matmul(
                        ps[:, 0:256],
                        xT[:, c, kb * 128:(kb + 1) * 128],
                        wv_sb[:, c, :],
                        start=(c == 0),
                        stop=(not with_qkv_bias and c == NCH - 1),
                    )
                if with_qkv_bias:
                    nc.tensor.matmul(
                        ps[:, 0:256], ones[0:1, 0:128], bv_sb[:],
                        start=False, stop=True,
                    )
                nc.vector.tensor_copy(
                    vsb[:, kb, :, 0:64],
                    ps[:, 0:256].rearrange("p (h c) -> p h c", h=HPC),
                )

            def s_exp(qb, pi, kb):
                qdb, kdb = pi, 2 + pi
                s01 = psS.tile([128, 1024], F32, tag="s01", name=f"s{qb}_{pi}_{kb}")
                nc.tensor.matmul(
                    s01[:, 0:512],
                    qkt[kdb][0:64, kb * 128:(kb + 1) * 128],
                    qkt[qdb][0:64, qb * 512:(qb + 1) * 512],
                )
                nc.tensor.matmul(
                    s01[:, 512:1024],
                    qkt[kdb][64:128, kb * 128:(kb + 1) * 128],
                    qkt[qdb][64:128, qb * 512:(qb + 1) * 512],
                )
                p01 = ptp.tile([128, 1024], BF16, tag="p01", name=f"p{qb}_{pi}_{kb}")
                nc.scalar.activation(
                    p01[:], s01[:], EXP, bias=bs_sb[:, kb:kb + 1], scale=SCALE
                )
                return p01

            osAB = {}

            def d_pair(qb, pi, kb, p01):
                if kb == 0:
                    osAB[(qb, pi)] = (
                        psO.tile([65, 512], F32, tag="osA", name=f"osA{qb}_{pi}"),
                        psO.tile([65, 512], F32, tag="osB", name=f"osB{qb}_{pi}"),
                    )
                osA, osB = osAB[(qb, pi)]
                nc.tensor.matmul(
                    osA[:], vsb[:, kb, 2 * pi, :], p01[:, 0:512],
                    start=(kb == 0), stop=(kb == NKB - 1),
                )
                nc.tensor.matmul(
                    osB[:], vsb[:, kb, 2 * pi + 1, :], p01[:, 512:1024],
                    start=(kb == 0), stop=(kb == NKB - 1),
                )

            stg_tiles = {}
            dens_tiles = {}

            def stage_out(qb, pi):
                # Drain O'^T psum accumulators -> SBUF promptly (frees psO for
                # the next group); row 64 holds the softmax denominators,
                # which a small SBUF->SBUF DMA shifts to partition 0 (DVE
                # lane ops cannot cross partitions on HW).
                stg = stgp.tile([65, 2, 512], F32, tag="stg", name=f"stg{qb}_{pi}")
                stg_tiles[(qb, pi)] = stg
                osA, osB = osAB.pop((qb, pi))
                nc.vector.tensor_copy(stg[:, 0, :], osA[:])
                nc.vector.tensor_copy(stg[:, 1, :], osB[:])
                dens = stgp.tile([1, 2, 512], F32, tag="dens", name=f"dn{qb}_{pi}")
                dens_tiles[(qb, pi)] = dens
                nc.sync.dma_start(dens[:], stg[64:65, :, :])

            def norm_group(qb, pi):
                # 1/denominators for this head pair (fast approx on DVE,
                # ~18 correct bits), broadcast across partitions on the idle
                # GPSIMD engine, then the normalize-multiply on DVE (bf16
                # out). Odd heads shift to partitions 64..127 via a small
                # SBUF->SBUF DMA.
                stg = stg_tiles[(qb, pi)]
                dens = dens_tiles.pop((qb, pi))
                rec = rcp.tile([1, 2, 512], F32, tag="rec", name=f"rec{qb}_{pi}")
                nc.vector.reciprocal_approx_fast(rec[:], dens[:])
                cols = slice(qb * 512, (qb + 1) * 512)
                for parity in range(2):
                    rbc = rbp.tile([64, 512], F32, tag="rbc",
                                   name=f"rbc{qb}_{pi}_{parity}")
                    nc.gpsimd.partition_broadcast(rbc[:], rec[0:1, parity, :])
                    if parity == 0:
                        nc.vector.tensor_mul(
                            ot[pi][0:64, cols], stg[0:64, parity, :], rbc[:]
                        )
                    else:
                        stag = rcp.tile([64, 512], BF16, tag="stag",
                                        name=f"st{qb}_{pi}")
                        nc.vector.tensor_mul(stag[:], stg[0:64, parity, :], rbc[:])
                        nc.sync.dma_start(ot[pi][64:128, cols], stag[:])
                stg_tiles.pop((qb, pi))

            def e_unit(tb, tail=False):
                # same-lhsT matmuls back-to-back (ot[0] pair, then ot[1]
                # pair). In the tail, stage the psum on the by-then-idle ACT
                # engine and split the y DMA across two queues so the drain
                # overlaps the remaining compute.
                ysb = ypool.tile([128, D], F32, tag="ysb", name=f"ysb{tb}")
                yps = [
                    psG.tile([128, 512], F32, tag="gp", name=f"yps{tb}_{nb}")
                    for nb in range(2)
                ]
                for pi in range(2):
                    for nb in range(2):
                        nc.tensor.matmul(
                            yps[nb][:],
                            ot[pi][:, tb * 128:(tb + 1) * 128],
                            wo_sb[:, pi, nb * 512:(nb + 1) * 512],
                            start=(pi == 0), stop=(pi == 1),
                        )
                for nb in range(2):
                    if tail:
                        nc.scalar.activation(
                            ysb[:, nb * 512:(nb + 1) * 512], yps[nb][:],
                            mybir.ActivationFunctionType.Copy,
                        )
                    else:
                        nc.vector.tensor_copy(
                            ysb[:, nb * 512:(nb + 1) * 512], yps[nb][:]
                        )
                if tail:
                    nc.sync.dma_start(y_rows[tb][:, 0:512], ysb[:, 0:512])
                    nc.scalar.dma_start(y_rows[tb][:, 512:1024], ysb[:, 512:1024])
                else:
                    nc.sync.dma_start(y_rows[tb], ysb[:])

            # ---------------- fill scheduling ----------------
            # Filler tasks (cost estimate ns, emit closure), deadline-ordered.
            fills = []

            def push(cost, fn):
                fills.append((cost, fn))

            # Deadline order: K-pair0 per tb feeds (qb,0) groups at kb=4tb;
            # Q/K for pi=1 by slot 16; B(kb) before lagged D; Q(tb) of later
            # q-blocks by their windows. norm/E tasks are appended dynamically.
            push(2800, lambda: a_unit(1, 2))
            push(2800, lambda: a_unit(0, 1))
            push(2800, lambda: a_unit(0, 3))
            push(2800, lambda: a_unit(2, 2))
            for kb in range(4):
                push(2600, lambda kb=kb: b_unit(kb))
            push(2800, lambda: a_unit(3, 2))
            push(2800, lambda: a_unit(1, 3))
            for kb in range(4, 8):
                push(2600, lambda kb=kb: b_unit(kb))
            push(2800, lambda: a_unit(1, 0))
            push(2800, lambda: a_unit(2, 3))
            for kb in range(8, 12):
                push(2600, lambda kb=kb: b_unit(kb))
            push(2800, lambda: a_unit(2, 0))
            push(2800, lambda: a_unit(3, 3))
            for kb in range(12, 16):
                push(2600, lambda kb=kb: b_unit(kb))
            push(2800, lambda: a_unit(1, 1))
            push(2800, lambda: a_unit(3, 0))
            push(2800, lambda: a_unit(2, 1))
            push(2800, lambda: a_unit(3, 1))

            fill_pos = [0]

            def pop_fills(budget):
                while fill_pos[0] < len(fills) and budget > 0:
                    cost, fn = fills[fill_pos[0]]
                    fn()
                    budget -= cost
                    fill_pos[0] += 1

            # ---------------- main schedule ----------------
            groups = [(qb, pi) for qb in range(NQB) for pi in range(2)]
            p_tiles = {}

            # prologue: Q-pair0 + K-pair0 of tb0 — just enough for G0
            a_unit(0, 0)
            a_unit(0, 2)

            for gi, (qb, pi) in enumerate(groups):
                for kb in range(NKB):
                    p_tiles[(qb, pi, kb)] = s_exp(qb, pi, kb)
                    if gi > 0:
                        pqb, ppi = groups[gi - 1]
                        d_pair(pqb, ppi, kb, p_tiles.pop((pqb, ppi, kb)))
                        budget = 500 if gi < 4 else 700
                    else:
                        budget = 1100
                    pop_fills(budget)
                    if gi > 0 and kb == NKB - 1:
                        pqb, ppi = groups[gi - 1]
                        stage_out(pqb, ppi)
                        push(2200, lambda q=pqb, p=ppi: norm_group(q, p))
                        if ppi == 1:
                            for tbl in range(4 * pqb, 4 * pqb + 4):
                                push(1800, lambda t=tbl: e_unit(t))

            # drain: last group's D with short lag, then its epilogue
            lqb, lpi = groups[-1]
            for kb in range(NKB):
                d_pair(lqb, lpi, kb, p_tiles.pop((lqb, lpi, kb)))
                pop_fills(700)
            stage_out(lqb, lpi)
            pop_fills(10 ** 9)
            norm_group(lqb, lpi)
            for tbl in range(4 * lqb, 4 * lqb + 4):
                e_unit(tbl, tail=True)

    nc.compile()
    return nc


def _get_nc(with_qkv_bias=False):
    key = ("nc", with_qkv_bias)
    if key not in _NC_CACHE:
        _NC_CACHE[key] = _build_nc(with_qkv_bias)
    return _NC_CACHE[key]


def _make_in_maps(x, boundary_score, W_qkv, b_qkv, W_out):
    import ml_dtypes

    BF = ml_dtypes.bfloat16
    x = np.asarray(x, np.float32)
    boundary_score = np.asarray(boundary_score, np.float32)
    W_qkv = np.asarray(W_qkv, np.float32)
    b_qkv = np.asarray(b_qkv, np.float32)
    W_out = np.asarray(W_out, np.float32)

    Wq, Wk, Wv = W_qkv[:, :D], W_qkv[:, D:2 * D], W_qkv[:, 2 * D:]
    bq, bk, bv = b_qkv[:D], b_qkv[D:2 * D], b_qkv[2 * D:]
    ones = np.ones((1, 512), BF)
    onesr = np.ones((1, 64), np.float32)
    xts = [np.ascontiguousarray(x[b].T).astype(BF) for b in range(x.shape[0])]

    in_maps = []
    for c in range(8):
        b, g = divmod(c, 4)
        lo, hi = 256 * g, 256 * (g + 1)
        wqk = np.ascontiguousarray(
            np.concatenate([Wq[:, lo:hi], Wk[:, lo:hi]], axis=1)
        ).astype(BF)
        bqk = np.concatenate([bq[lo:hi], bk[lo:hi]])[None].astype(BF)
        wv = np.ascontiguousarray(Wv[:, lo:hi]).astype(BF)
        bvv = np.ascontiguousarray(bv[lo:hi][None]).astype(BF)
        wo = np.ascontiguousarray(W_out[lo:hi, :]).astype(BF)
        bs = np.ascontiguousarray(
            (boundary_score[b] * BIAS_COEF).reshape(NKB, 128).T
        )
        in_maps.append(
            dict(
                xt=xts[b], wqk=wqk, bqk=np.ascontiguousarray(bqk),
                wv=wv, bv=bvv, wo=wo, bs=bs, ones=ones, onesr=onesr,
            )
        )
    return in_maps


def kernel(x, boundary_score, W_qkv, b_qkv, W_out, b_out):
    from concourse.bass_utils import run_bass_kernel_spmd

    x = np.asarray(x, np.float32)
    B = x.shape[0]
    in_maps = _make_in_maps(x, boundary_score, W_qkv, b_qkv, W_out)
    nc = _get_nc(with_qkv_bias=bool(np.any(np.asarray(b_qkv))))
    res = run_bass_kernel_spmd(nc, in_maps, list(range(8))).results
    out = np.zeros((B, T, D), np.float32)
    for c in range(8):
        out[c // 4] += res[c]["y"]
    out += np.asarray(b_out, np.float32)
    return out
